# revision 16
# baseline (speedup 1.0000x reference)
"""Trainium2 Bass kernel for nn_DependencyLinearLayer.

Math (collapsed-H reformulation of the reference):
  out[b,i,c,j] = dep_logits[dg[b,i,j], c] + s_log[b,i,c] + t_log[b,j,c] + cls_b[c]
where
  dep_logits = dep_emb @ w_d.T                  [48, 12]
  s_log      = x @ (w_s @ s_fc_w).T + w_s@s_fc_b  (combined-weight form)
  t_log      = x @ (w_t @ t_fc_w).T + w_t@t_fc_b
  w_s, w_t, w_d = cls_w[:, :H], cls_w[:, H:2H], cls_w[:, 2H:]

Sharding: 8 cores; core n handles batch b = n//2 and i-rows [128*(n%2), 128*(n%2)+128).

The per-element 48-entry table lookup runs on GPSIMD via ap_gather with a
PAIRED table: table2[48*a+b] = (T[a], T[b]) so each gather index produces two
consecutive output j's, halving the Q7 read-command count (the bottleneck).
All floating-point math (projections, table construction, broadcast adds)
runs on-device (PE/ACT/DVE); the host only reshapes/shards inputs.
"""

import os
import sys

import numpy as np

for _p in ("/opt/trn_rl_repo",):
    if _p not in sys.path:
        sys.path.insert(0, _p)

import concourse.bass as bass  # noqa: E402
import concourse.tile as tile  # noqa: E402
from concourse import bacc, mybir  # noqa: E402
from concourse.tile import ScopedClock  # noqa: E402

B, L, IN, H, C, NDEP = 4, 256, 768, 256, 12, 48
NCORES = 8
RPC = L // 2  # i-rows per core (128)
NINST = 8    # ap_gather instructions per core (2 i-rows x 8 groups each)

_MAX_TAIL_WAITS = 1


def _patched_drain_and_barrier(self, tick_clock, wait_clock):
    # The walrus build in this image rejects >1 sync-wait on one CTRL
    # instruction; split the kernel-tail drain waits across nops.
    drain_inst = self.nc.sync.drain()
    wait_clock.add_sem_waits(
        drain_inst.ins, ScopedClock({None: tick_clock.global_clock})
    )
    sync_info = drain_inst.ins.sync_info
    if sync_info is not None and len(sync_info.on_wait) > _MAX_TAIL_WAITS:
        waits = list(sync_info.on_wait)
        sync_info.on_wait = waits[:_MAX_TAIL_WAITS]
        rest = waits[_MAX_TAIL_WAITS:]
        while rest:
            chunk, rest = rest[:_MAX_TAIL_WAITS], rest[_MAX_TAIL_WAITS:]
            nop = self.nc.sync.nop(nofuse=True, hint="tail_drain_split").ins
            nop.sync_info = mybir.SyncInfo(on_wait=chunk, on_update=[])
    self.nc.all_engine_barrier()
    assert self.sems is not None
    popped = self.nc._tile_sem_poison_stack.pop()
    assert popped is self._sem_poison
    self.nc.clear_and_free_semaphores(list(self.sems.allocated().values()))
    self.nc.all_engine_barrier()


tile.TileContext._drain_and_barrier = _patched_drain_and_barrier

_PROGRAM = None

# raw block order (block=2u+h, g) -> i_loc = 16u + 8h + g; _ROWPERM[i_loc] = block*8+g
_ROWPERM = np.zeros(RPC, dtype=np.int64)
for _u in range(NINST):
    for _h in range(2):
        for _g in range(8):
            _ROWPERM[16 * _u + 8 * _h + _g] = (2 * _u + _h) * 8 + _g


def build_program():
    f32 = mybir.dt.float32
    nc = bacc.Bacc("TRN2", target_bir_lowering=False, debug=False)

    xbT = nc.declare_dram_parameter("xbT", [IN, L], f32, isOutput=False)
    xsT = nc.declare_dram_parameter("xsT", [IN, RPC], f32, isOutput=False)
    dgw = nc.declare_dram_parameter("dgw", [128, 128], mybir.dt.int16, isOutput=False)
    sfw = nc.declare_dram_parameter("sfw", [H, IN], f32, isOutput=False)
    tfw = nc.declare_dram_parameter("tfw", [H, IN], f32, isOutput=False)
    wsT = nc.declare_dram_parameter("wsT", [H, C], f32, isOutput=False)
    wtT = nc.declare_dram_parameter("wtT", [H, C], f32, isOutput=False)
    wdT = nc.declare_dram_parameter("wdT", [H, C], f32, isOutput=False)
    sfb = nc.declare_dram_parameter("sfb", [H, 1], f32, isOutput=False)
    tfb = nc.declare_dram_parameter("tfb", [H, 1], f32, isOutput=False)
    depT = nc.declare_dram_parameter("depT", [H, NDEP], f32, isOutput=False)
    clbT = nc.declare_dram_parameter("clbT", [C, 1], f32, isOutput=False)
    repmat = nc.declare_dram_parameter("repmat", [C, 128], f32, isOutput=False)
    out_d = nc.declare_dram_parameter("out", [RPC * 16, L], f32, isOutput=True)

    Copy = mybir.ActivationFunctionType.Copy

    with tile.TileContext(nc) as tc:
        with (
            tc.tile_pool(name="const", bufs=1) as cp,
            tc.tile_pool(name="gpool", bufs=3) as gp,
            tc.tile_pool(name="psum", bufs=1, space="PSUM") as pp,
            tc.tile_pool(name="psum2", bufs=1, space="PSUM") as pp2,
        ):
            # ---- input loads (straight, contiguous) ----
            warm_tab = cp.tile([128, 4], f32, tag="warm_tab")
            warm_out = cp.tile([128, 16], f32, tag="warm_out")
            sfw_t = cp.tile([128, 2 * IN], f32, tag="sfw_t")
            tfw_t = cp.tile([128, 2 * IN], f32, tag="tfw_t")
            for h0 in range(2):
                nc.sync.dma_start(sfw_t[:, h0 * IN:(h0 + 1) * IN], sfw[h0 * 128:(h0 + 1) * 128, :])
                nc.sync.dma_start(tfw_t[:, h0 * IN:(h0 + 1) * IN], tfw[h0 * 128:(h0 + 1) * 128, :])
            wsT_t = cp.tile([128, 2 * C], f32, tag="wsT_t")
            wtT_t = cp.tile([128, 2 * C], f32, tag="wtT_t")
            wdT_t = cp.tile([128, 2 * C], f32, tag="wdT_t")
            depT_t = cp.tile([128, 2 * NDEP], f32, tag="depT_t")
            sfb_t = cp.tile([128, 2], f32, tag="sfb_t")
            tfb_t = cp.tile([128, 2], f32, tag="tfb_t")
            for h0 in range(2):
                sl = slice(h0 * 128, (h0 + 1) * 128)
                nc.scalar.dma_start(wsT_t[:, h0 * C:(h0 + 1) * C], wsT[sl, :])
                nc.scalar.dma_start(wtT_t[:, h0 * C:(h0 + 1) * C], wtT[sl, :])
                nc.scalar.dma_start(wdT_t[:, h0 * C:(h0 + 1) * C], wdT[sl, :])
                nc.scalar.dma_start(depT_t[:, h0 * NDEP:(h0 + 1) * NDEP], depT[sl, :])
                nc.scalar.dma_start(sfb_t[:, h0:h0 + 1], sfb[sl, :])
                nc.scalar.dma_start(tfb_t[:, h0:h0 + 1], tfb[sl, :])
            with tc.high_priority():
                warm_src = wdT_t[:, 0:4]
                nc.vector.tensor_scalar_mul(warm_tab[:], warm_src, 0.0)
                nc.gpsimd.ap_gather(
                    warm_out[:], warm_tab[:],
                    warm_tab[:, 0:1].bitcast(mybir.dt.int16)[:, 0:1],
                    channels=128, num_elems=4, d=1, num_idxs=16,
                )
            x_t = cp.tile([128, 6 * L], f32, tag="x_t")
            for m in range(6):
                nc.sync.dma_start(x_t[:, m * L:(m + 1) * L], xbT[m * 128:(m + 1) * 128, :])
            xs_t = cp.tile([128, 6 * RPC], f32, tag="xs_t")
            for m in range(6):
                nc.sync.dma_start(xs_t[:, m * RPC:(m + 1) * RPC], xsT[m * 128:(m + 1) * 128, :])
            clbT_t = cp.tile([C, 1], f32, tag="clbT_t")
            nc.scalar.dma_start(clbT_t[:], clbT[:])
            rep_t = cp.tile([C, 128], f32, tag="rep_t")
            nc.sync.dma_start(rep_t[:], repmat[:])
            dgw_t = cp.tile([128, 128], mybir.dt.int16, tag="dgw_t")
            nc.scalar.dma_start(dgw_t[:], dgw[:])

            # ---- dep_logitsT [12, 48] + bias + table2: gather-critical ----
            with tc.high_priority():
                pd = pp.tile([C, NDEP], f32, tag="pd")
                for h0 in range(2):
                    nc.tensor.matmul(
                        pd[:],
                        wdT_t[:, h0 * C:(h0 + 1) * C],
                        depT_t[:, h0 * NDEP:(h0 + 1) * NDEP],
                        start=(h0 == 0), stop=(h0 == 1),
                    )
                pb = pp.tile([C, 1], f32, tag="pb")
                nc.tensor.matmul(pb[:], wsT_t[:, 0:C], sfb_t[:, 0:1], start=True, stop=False)
                nc.tensor.matmul(pb[:], wsT_t[:, C:2 * C], sfb_t[:, 1:2], start=False, stop=False)
                nc.tensor.matmul(pb[:], wtT_t[:, 0:C], tfb_t[:, 0:1], start=False, stop=False)
                nc.tensor.matmul(pb[:], wtT_t[:, C:2 * C], tfb_t[:, 1:2], start=False, stop=True)
                bias_t = cp.tile([C, 1], f32, tag="bias_t")
                nc.vector.tensor_add(bias_t[:], pb[:], clbT_t[:])

            with tc.high_priority():
                midbufD = cp.tile([C, NDEP], f32, tag="midbufD")
                nc.scalar.activation(midbufD[:], pd[:],
                                     mybir.ActivationFunctionType.Identity, bias=bias_t[:])
                t16_t = cp.tile([128, NDEP], f32, tag="t16_t")
                nc.vector.tensor_scalar_mul(t16_t[:], depT_t[:, 0:NDEP], 0.0)
                for g in range(8):
                    nc.scalar.dma_start(t16_t[16 * g:16 * g + C, :], midbufD[:])

                table2 = cp.tile([128, NDEP * NDEP * 2], f32, tag="table2")
                tv = table2[:].rearrange("p (a b t) -> p a b t", a=NDEP, b=NDEP, t=2)
                nc.vector.tensor_copy(
                    tv[:, :, :, 0], t16_t[:].unsqueeze(2).broadcast_to([128, NDEP, NDEP])
                )
                nc.scalar.copy(
                    tv[:, :, :, 1], t16_t[:].unsqueeze(1).broadcast_to([128, NDEP, NDEP])
                )

            # ---- combined weights W2[k, 0:12]=swT, [12:24]=twT ----
            w2_t = cp.tile([128, 6 * 24], f32, tag="w2_t")
            for m in range(6):
                pw = pp.tile([128, 24], f32, tag="pw")
                for h0 in range(2):
                    nc.tensor.matmul(
                        pw[:, 0:C],
                        sfw_t[:, h0 * IN + m * 128: h0 * IN + (m + 1) * 128],
                        wsT_t[:, h0 * C:(h0 + 1) * C],
                        start=(h0 == 0), stop=(h0 == 1),
                    )
                for h0 in range(2):
                    nc.tensor.matmul(
                        pw[:, C:2 * C],
                        tfw_t[:, h0 * IN + m * 128: h0 * IN + (m + 1) * 128],
                        wtT_t[:, h0 * C:(h0 + 1) * C],
                        start=(h0 == 0), stop=(h0 == 1),
                    )
                nc.vector.tensor_copy(w2_t[:, m * 24:(m + 1) * 24], pw[:])


            # ---- projections: s_logT [12, 256] and t_logT [12, 256] ----
            ps = pp2.tile([C, RPC], f32, tag="ps")
            pt = pp2.tile([C, L], f32, tag="pt")
            for m in range(6):
                nc.tensor.matmul(
                    ps[:], w2_t[:, m * 24: m * 24 + C], xs_t[:, m * RPC:(m + 1) * RPC],
                    start=(m == 0), stop=(m == 5),
                )
            for m in range(6):
                nc.tensor.matmul(
                    pt[:], w2_t[:, m * 24 + C: m * 24 + 2 * C], x_t[:, m * L:(m + 1) * L],
                    start=(m == 0), stop=(m == 5),
                )



            # ---- t/s projections epilogue ----
            midbufT = cp.tile([C, L], f32, tag="midbufT")
            nc.scalar.activation(midbufT[:], pt[:], Copy)
            slog_t = cp.tile([C, RPC], f32, tag="slog_t")
            nc.scalar.activation(slog_t[:], ps[:], Copy)
            prepT = pp2.tile([128, L], f32, tag="prepT")
            nc.tensor.matmul(prepT[:], rep_t[:], midbufT[:], start=True, stop=True)
            t16u16 = cp.tile([128, L], f32, tag="t16u16")
            nc.vector.tensor_copy(t16u16[:], prepT[:])

            # ---- S_all [128, 16]: per-(instruction, half) per-partition scalars ----
            # S_all[16g+c, 2u+h] = s_logT[c, 16u+8h+g]
            s_all = cp.tile([128, 16], f32, tag="s_all")
            nc.vector.tensor_scalar_mul(s_all[:], sfw_t[:, 0:16], 0.0)
            for g in range(8):
                nc.sync.dma_start(
                    s_all[16 * g:16 * g + C, :],
                    slog_t[0:C, g:g + 121:8],
                )

            # ---- gathers + fused adds + stores ----
            u16 = t16u16[:]
            for u in range(NINST):
                g_t = gp.tile([128, 2 * L], f32, tag="g_t")
                nc.gpsimd.ap_gather(
                    g_t[:].rearrange("p (k t) -> p k t", t=2),
                    table2[:].rearrange("p (e t) -> p e t", t=2),
                    dgw_t[:, 16 * u:16 * (u + 1)],
                    channels=128, num_elems=NDEP * NDEP, d=2, num_idxs=256,
                )
                for h in range(2):
                    nc.vector.affine_then_add(
                        g_t[:, h * L:(h + 1) * L],
                        g_t[:, h * L:(h + 1) * L],
                        u16,
                        1.0,
                        s_all[:, 2 * u + h:2 * u + h + 1],
                    )
                nc.sync.dma_start(
                    out_d[2 * u * 128:(2 * u + 2) * 128, :].rearrange(
                        "(h p) j -> p h j", h=2
                    ),
                    g_t[:].rearrange("p (h j) -> p h j", h=2),
                )

    nc.compile()
    return nc


def _marshal_core(n, input_tensor, dg, consts):
    b, half = n // 2, n % 2
    i0 = half * RPC
    dgb = dg[b]
    # paired indices, wrapped per 16-partition group:
    # instruction u, group g -> stream of 256: 128 pairs of row 16u+g,
    # then 128 pairs of row 16u+8+g; stream[k] at [16g + k%16, 16u + k//16].
    pairs = (dgb[:, 0::2] * NDEP + dgb[:, 1::2]).astype(np.int16)  # [L, 128]
    dgw = np.empty((128, 128), dtype=np.int16)
    for u in range(8):
        for g in range(8):
            stream = np.concatenate(
                [pairs[i0 + 16 * u + g], pairs[i0 + 16 * u + 8 + g]]
            )  # [256]
            dgw[16 * g:16 * (g + 1), 16 * u:16 * (u + 1)] = stream.reshape(16, 16).T
    m = {
        "xbT": np.ascontiguousarray(input_tensor[b].T),
        "xsT": np.ascontiguousarray(input_tensor[b, i0:i0 + RPC].T),
        "dgw": dgw,
    }
    m.update(consts)
    return m


def kernel(input_tensor, dependency_graph, s_fc_w, s_fc_b, t_fc_w, t_fc_b,
           dep_emb, cls_w, cls_b):
    global _PROGRAM
    from concourse.bass_utils import run_bass_kernel_spmd

    input_tensor = np.asarray(input_tensor, dtype=np.float32)
    dg = np.asarray(dependency_graph)
    out_dtype = np.float32

    consts = {
        "sfw": np.ascontiguousarray(np.asarray(s_fc_w, np.float32)),
        "tfw": np.ascontiguousarray(np.asarray(t_fc_w, np.float32)),
        "wsT": np.ascontiguousarray(np.asarray(cls_w, np.float32)[:, 0:H].T),
        "wtT": np.ascontiguousarray(np.asarray(cls_w, np.float32)[:, H:2 * H].T),
        "wdT": np.ascontiguousarray(np.asarray(cls_w, np.float32)[:, 2 * H:].T),
        "sfb": np.asarray(s_fc_b, np.float32).reshape(H, 1).copy(),
        "tfb": np.asarray(t_fc_b, np.float32).reshape(H, 1).copy(),
        "depT": np.ascontiguousarray(np.asarray(dep_emb, np.float32).T),
        "clbT": np.asarray(cls_b, np.float32).reshape(C, 1).copy(),
    }
    # repmat[c, 16g+c'] = (c' == c)
    rm = np.zeros((C, 128), dtype=np.float32)
    for g in range(8):
        rm[np.arange(C), 16 * g + np.arange(C)] = 1.0
    consts["repmat"] = rm

    if _PROGRAM is None:
        _PROGRAM = build_program()
    nc = _PROGRAM

    in_maps = [_marshal_core(n, input_tensor, dg, consts) for n in range(NCORES)]
    trace = bool(int(os.environ.get("KERNEL_PROFILE", "0")))
    res = run_bass_kernel_spmd(
        nc, in_maps, core_ids=list(range(NCORES)), trace=trace
    )
    if trace and res.exec_time_ns is not None:
        print(f"HW exec time: {res.exec_time_ns} ns")

    out = np.empty((B, L, C, L), dtype=out_dtype)
    for n in range(NCORES):
        b, half = n // 2, n % 2
        i0 = half * RPC
        # raw flat row = (2u+h)*128 + 16g + c ; i_loc = 16u + 8h + g
        raw = res.results[n]["out"].reshape(2 * NINST, 8, 16, L)  # [block=2u+h, g, c16, j]
        out[b, i0:i0 + RPC] = raw[:, :, :C, :].reshape(2 * NINST * 8, C, L)[_ROWPERM]
    return out


# revision 17
# speedup vs baseline: 1.0538x; 1.0538x over previous
"""Trainium2 Bass kernel for nn_DependencyLinearLayer.

Math (collapsed-H reformulation of the reference):
  out[b,i,c,j] = dep_logits[dg[b,i,j], c] + s_log[b,i,c] + t_log[b,j,c] + cls_b[c]
where
  dep_logits = dep_emb @ w_d.T                  [48, 12]
  s_log      = x @ (w_s @ s_fc_w).T + w_s@s_fc_b  (combined-weight form)
  t_log      = x @ (w_t @ t_fc_w).T + w_t@t_fc_b
  w_s, w_t, w_d = cls_w[:, :H], cls_w[:, H:2H], cls_w[:, 2H:]

Sharding: 8 cores; core n handles batch b = n//2 and i-rows [128*(n%2), 128*(n%2)+128).

The per-element 48-entry table lookup runs on GPSIMD via ap_gather with a
PAIRED table: table2[48*a+b] = (T[a], T[b]) so each gather index produces two
consecutive output j's, halving the Q7 read-command count (the bottleneck).
All floating-point math (projections, table construction, broadcast adds)
runs on-device (PE/ACT/DVE); the host only reshapes/shards inputs.
"""

import os
import sys

import numpy as np

for _p in ("/opt/trn_rl_repo",):
    if _p not in sys.path:
        sys.path.insert(0, _p)

import concourse.bass as bass  # noqa: E402
import concourse.tile as tile  # noqa: E402
from concourse import bacc, mybir  # noqa: E402
from concourse.tile import ScopedClock  # noqa: E402

B, L, IN, H, C, NDEP = 4, 256, 768, 256, 12, 48
NCORES = 8
RPC = L // 2  # i-rows per core (128)
NINST = 8    # ap_gather instructions per core (2 i-rows x 8 groups each)

_MAX_TAIL_WAITS = 1


def _patched_drain_and_barrier(self, tick_clock, wait_clock):
    # The walrus build in this image rejects >1 sync-wait on one CTRL
    # instruction; split the kernel-tail drain waits across nops.
    drain_inst = self.nc.sync.drain()
    wait_clock.add_sem_waits(
        drain_inst.ins, ScopedClock({None: tick_clock.global_clock})
    )
    sync_info = drain_inst.ins.sync_info
    if sync_info is not None and len(sync_info.on_wait) > _MAX_TAIL_WAITS:
        waits = list(sync_info.on_wait)
        sync_info.on_wait = waits[:_MAX_TAIL_WAITS]
        rest = waits[_MAX_TAIL_WAITS:]
        while rest:
            chunk, rest = rest[:_MAX_TAIL_WAITS], rest[_MAX_TAIL_WAITS:]
            nop = self.nc.sync.nop(nofuse=True, hint="tail_drain_split").ins
            nop.sync_info = mybir.SyncInfo(on_wait=chunk, on_update=[])
    self.nc.all_engine_barrier()
    assert self.sems is not None
    popped = self.nc._tile_sem_poison_stack.pop()
    assert popped is self._sem_poison
    self.nc.clear_and_free_semaphores(list(self.sems.allocated().values()))
    self.nc.all_engine_barrier()


tile.TileContext._drain_and_barrier = _patched_drain_and_barrier

_PROGRAM = None

# raw block order (block=2u+h, g) -> i_loc = 16u + 8h + g; _ROWPERM[i_loc] = block*8+g
_ROWPERM = np.zeros(RPC, dtype=np.int64)
for _u in range(NINST):
    for _h in range(2):
        for _g in range(8):
            _ROWPERM[16 * _u + 8 * _h + _g] = (2 * _u + _h) * 8 + _g


def build_program():
    f32 = mybir.dt.float32
    nc = bacc.Bacc("TRN2", target_bir_lowering=False, debug=False)

    xbT = nc.declare_dram_parameter("xbT", [IN, L], f32, isOutput=False)
    xsT = nc.declare_dram_parameter("xsT", [IN, RPC], f32, isOutput=False)
    dgw = nc.declare_dram_parameter("dgw", [128, 128], mybir.dt.int16, isOutput=False)
    sfw = nc.declare_dram_parameter("sfw", [H, IN], f32, isOutput=False)
    tfw = nc.declare_dram_parameter("tfw", [H, IN], f32, isOutput=False)
    wsT = nc.declare_dram_parameter("wsT", [H, C], f32, isOutput=False)
    wtT = nc.declare_dram_parameter("wtT", [H, C], f32, isOutput=False)
    wdT = nc.declare_dram_parameter("wdT", [H, C], f32, isOutput=False)
    sfb = nc.declare_dram_parameter("sfb", [H, 1], f32, isOutput=False)
    tfb = nc.declare_dram_parameter("tfb", [H, 1], f32, isOutput=False)
    depT = nc.declare_dram_parameter("depT", [H, NDEP], f32, isOutput=False)
    clbT = nc.declare_dram_parameter("clbT", [C, 1], f32, isOutput=False)
    repmat = nc.declare_dram_parameter("repmat", [C, 128], f32, isOutput=False)
    out_d = nc.declare_dram_parameter("out", [RPC * 16, L], f32, isOutput=True)

    Copy = mybir.ActivationFunctionType.Copy

    with tile.TileContext(nc) as tc:
        with (
            tc.tile_pool(name="const", bufs=1) as cp,
            tc.tile_pool(name="gpool", bufs=3) as gp,
            tc.tile_pool(name="psum", bufs=1, space="PSUM") as pp,
            tc.tile_pool(name="psum2", bufs=1, space="PSUM") as pp2,
        ):
            # ---- input loads (straight, contiguous) ----
            warm_tab = cp.tile([128, 4], f32, tag="warm_tab")
            warm_out = cp.tile([128, 16], f32, tag="warm_out")
            sfw_t = cp.tile([128, 2 * IN], f32, tag="sfw_t")
            tfw_t = cp.tile([128, 2 * IN], f32, tag="tfw_t")
            for h0 in range(2):
                nc.sync.dma_start(sfw_t[:, h0 * IN:(h0 + 1) * IN], sfw[h0 * 128:(h0 + 1) * 128, :])
                nc.sync.dma_start(tfw_t[:, h0 * IN:(h0 + 1) * IN], tfw[h0 * 128:(h0 + 1) * 128, :])
            wsT_t = cp.tile([128, 2 * C], f32, tag="wsT_t")
            wtT_t = cp.tile([128, 2 * C], f32, tag="wtT_t")
            wdT_t = cp.tile([128, 2 * C], f32, tag="wdT_t")
            depT_t = cp.tile([128, 2 * NDEP], f32, tag="depT_t")
            sfb_t = cp.tile([128, 2], f32, tag="sfb_t")
            tfb_t = cp.tile([128, 2], f32, tag="tfb_t")
            for h0 in range(2):
                sl = slice(h0 * 128, (h0 + 1) * 128)
                nc.scalar.dma_start(wsT_t[:, h0 * C:(h0 + 1) * C], wsT[sl, :])
                nc.scalar.dma_start(wtT_t[:, h0 * C:(h0 + 1) * C], wtT[sl, :])
                nc.scalar.dma_start(wdT_t[:, h0 * C:(h0 + 1) * C], wdT[sl, :])
                nc.scalar.dma_start(depT_t[:, h0 * NDEP:(h0 + 1) * NDEP], depT[sl, :])
                nc.scalar.dma_start(sfb_t[:, h0:h0 + 1], sfb[sl, :])
                nc.scalar.dma_start(tfb_t[:, h0:h0 + 1], tfb[sl, :])
            with tc.high_priority():
                warm_src = wdT_t[:, 0:4]
                nc.vector.tensor_scalar_mul(warm_tab[:], warm_src, 0.0)
                nc.gpsimd.ap_gather(
                    warm_out[:], warm_tab[:],
                    warm_tab[:, 0:1].bitcast(mybir.dt.int16)[:, 0:1],
                    channels=128, num_elems=4, d=1, num_idxs=16,
                )
            x_t = cp.tile([128, 6 * L], f32, tag="x_t")
            for m in range(6):
                nc.sync.dma_start(x_t[:, m * L:(m + 1) * L], xbT[m * 128:(m + 1) * 128, :])
            xs_t = cp.tile([128, 6 * RPC], f32, tag="xs_t")
            for m in range(6):
                nc.sync.dma_start(xs_t[:, m * RPC:(m + 1) * RPC], xsT[m * 128:(m + 1) * 128, :])
            clbT_t = cp.tile([C, 1], f32, tag="clbT_t")
            nc.scalar.dma_start(clbT_t[:], clbT[:])
            rep_t = cp.tile([C, 128], f32, tag="rep_t")
            nc.sync.dma_start(rep_t[:], repmat[:])
            dgw_t = cp.tile([128, 128], mybir.dt.int16, tag="dgw_t")
            nc.scalar.dma_start(dgw_t[:], dgw[:])

            # ---- dep_logitsT+biases [12, 48] in ONE accumulation group ----
            # pd[c,d] = sum_h wd[h,c]*depT[h,d] + ws@sfb + wt@tfb (bcast over d)
            with tc.high_priority():
                pd = pp.tile([C, NDEP], f32, tag="pd")
                nc.tensor.matmul(pd[:], wdT_t[:, 0:C], depT_t[:, 0:NDEP],
                                 start=True, stop=False)
                nc.tensor.matmul(pd[:], wdT_t[:, C:2 * C], depT_t[:, NDEP:2 * NDEP],
                                 start=False, stop=False)
                nc.tensor.matmul(pd[:], wsT_t[:, 0:C],
                                 sfb_t[:, 0:1].broadcast_to([128, NDEP]),
                                 start=False, stop=False)
                nc.tensor.matmul(pd[:], wsT_t[:, C:2 * C],
                                 sfb_t[:, 1:2].broadcast_to([128, NDEP]),
                                 start=False, stop=False)
                nc.tensor.matmul(pd[:], wtT_t[:, 0:C],
                                 tfb_t[:, 0:1].broadcast_to([128, NDEP]),
                                 start=False, stop=False)
                nc.tensor.matmul(pd[:], wtT_t[:, C:2 * C],
                                 tfb_t[:, 1:2].broadcast_to([128, NDEP]),
                                 start=False, stop=True)

            with tc.high_priority():
                midbufD = cp.tile([C, NDEP], f32, tag="midbufD")
                nc.scalar.activation(midbufD[:], pd[:],
                                     mybir.ActivationFunctionType.Identity, bias=clbT_t[:])
                t16_t = cp.tile([128, NDEP], f32, tag="t16_t")
                nc.vector.tensor_scalar_mul(t16_t[:], depT_t[:, 0:NDEP], 0.0)
                for g in range(8):
                    nc.scalar.dma_start(t16_t[16 * g:16 * g + C, :], midbufD[:])

                table2 = cp.tile([128, NDEP * NDEP * 2], f32, tag="table2")
                tv = table2[:].rearrange("p (a b t) -> p a b t", a=NDEP, b=NDEP, t=2)
                nc.vector.tensor_copy(
                    tv[:, :, :, 0], t16_t[:].unsqueeze(2).broadcast_to([128, NDEP, NDEP])
                )
                nc.scalar.copy(
                    tv[:, :, :, 1], t16_t[:].unsqueeze(1).broadcast_to([128, NDEP, NDEP])
                )

            # ---- combined weights W2[k, 0:12]=swT, [12:24]=twT ----
            w2_t = cp.tile([128, 6 * 24], f32, tag="w2_t")
            for m in range(6):
                pw = pp.tile([128, 24], f32, tag="pw")
                for h0 in range(2):
                    nc.tensor.matmul(
                        pw[:, 0:C],
                        sfw_t[:, h0 * IN + m * 128: h0 * IN + (m + 1) * 128],
                        wsT_t[:, h0 * C:(h0 + 1) * C],
                        start=(h0 == 0), stop=(h0 == 1),
                    )
                for h0 in range(2):
                    nc.tensor.matmul(
                        pw[:, C:2 * C],
                        tfw_t[:, h0 * IN + m * 128: h0 * IN + (m + 1) * 128],
                        wtT_t[:, h0 * C:(h0 + 1) * C],
                        start=(h0 == 0), stop=(h0 == 1),
                    )
                nc.vector.tensor_copy(w2_t[:, m * 24:(m + 1) * 24], pw[:])


            # ---- projections: s_logT [12, 256] and t_logT [12, 256] ----
            ps = pp2.tile([C, RPC], f32, tag="ps")
            pt = pp2.tile([C, L], f32, tag="pt")
            for m in range(6):
                nc.tensor.matmul(
                    ps[:], w2_t[:, m * 24: m * 24 + C], xs_t[:, m * RPC:(m + 1) * RPC],
                    start=(m == 0), stop=(m == 5),
                )
            for m in range(6):
                nc.tensor.matmul(
                    pt[:], w2_t[:, m * 24 + C: m * 24 + 2 * C], x_t[:, m * L:(m + 1) * L],
                    start=(m == 0), stop=(m == 5),
                )



            # ---- t/s projections epilogue ----
            midbufT = cp.tile([C, L], f32, tag="midbufT")
            nc.scalar.activation(midbufT[:], pt[:], Copy)
            slog_t = cp.tile([C, RPC], f32, tag="slog_t")
            nc.scalar.activation(slog_t[:], ps[:], Copy)
            prepT = pp2.tile([128, L], f32, tag="prepT")
            nc.tensor.matmul(prepT[:], rep_t[:], midbufT[:], start=True, stop=True)
            t16u16 = cp.tile([128, L], f32, tag="t16u16")
            nc.vector.tensor_copy(t16u16[:], prepT[:])

            # ---- S_all [128, 16]: per-(instruction, half) per-partition scalars ----
            # S_all[16g+c, 2u+h] = s_logT[c, 16u+8h+g]
            s_all = cp.tile([128, 16], f32, tag="s_all")
            nc.vector.tensor_scalar_mul(s_all[:], sfw_t[:, 0:16], 0.0)
            for g in range(8):
                nc.sync.dma_start(
                    s_all[16 * g:16 * g + C, :],
                    slog_t[0:C, g:g + 121:8],
                )

            # ---- gathers + fused adds + stores ----
            u16 = t16u16[:]
            for u in range(NINST):
                g_t = gp.tile([128, 2 * L], f32, tag="g_t")
                nc.gpsimd.ap_gather(
                    g_t[:].rearrange("p (k t) -> p k t", t=2),
                    table2[:].rearrange("p (e t) -> p e t", t=2),
                    dgw_t[:, 16 * u:16 * (u + 1)],
                    channels=128, num_elems=NDEP * NDEP, d=2, num_idxs=256,
                )
                for h in range(2):
                    nc.vector.affine_then_add(
                        g_t[:, h * L:(h + 1) * L],
                        g_t[:, h * L:(h + 1) * L],
                        u16,
                        1.0,
                        s_all[:, 2 * u + h:2 * u + h + 1],
                    )
                nc.sync.dma_start(
                    out_d[2 * u * 128:(2 * u + 2) * 128, :].rearrange(
                        "(h p) j -> p h j", h=2
                    ),
                    g_t[:].rearrange("p (h j) -> p h j", h=2),
                )

    nc.compile()
    return nc


def _marshal_core(n, input_tensor, dg, consts):
    b, half = n // 2, n % 2
    i0 = half * RPC
    dgb = dg[b]
    # paired indices, wrapped per 16-partition group:
    # instruction u, group g -> stream of 256: 128 pairs of row 16u+g,
    # then 128 pairs of row 16u+8+g; stream[k] at [16g + k%16, 16u + k//16].
    pairs = (dgb[:, 0::2] * NDEP + dgb[:, 1::2]).astype(np.int16)  # [L, 128]
    dgw = np.empty((128, 128), dtype=np.int16)
    for u in range(8):
        for g in range(8):
            stream = np.concatenate(
                [pairs[i0 + 16 * u + g], pairs[i0 + 16 * u + 8 + g]]
            )  # [256]
            dgw[16 * g:16 * (g + 1), 16 * u:16 * (u + 1)] = stream.reshape(16, 16).T
    m = {
        "xbT": np.ascontiguousarray(input_tensor[b].T),
        "xsT": np.ascontiguousarray(input_tensor[b, i0:i0 + RPC].T),
        "dgw": dgw,
    }
    m.update(consts)
    return m


def kernel(input_tensor, dependency_graph, s_fc_w, s_fc_b, t_fc_w, t_fc_b,
           dep_emb, cls_w, cls_b):
    global _PROGRAM
    from concourse.bass_utils import run_bass_kernel_spmd

    input_tensor = np.asarray(input_tensor, dtype=np.float32)
    dg = np.asarray(dependency_graph)
    out_dtype = np.float32

    consts = {
        "sfw": np.ascontiguousarray(np.asarray(s_fc_w, np.float32)),
        "tfw": np.ascontiguousarray(np.asarray(t_fc_w, np.float32)),
        "wsT": np.ascontiguousarray(np.asarray(cls_w, np.float32)[:, 0:H].T),
        "wtT": np.ascontiguousarray(np.asarray(cls_w, np.float32)[:, H:2 * H].T),
        "wdT": np.ascontiguousarray(np.asarray(cls_w, np.float32)[:, 2 * H:].T),
        "sfb": np.asarray(s_fc_b, np.float32).reshape(H, 1).copy(),
        "tfb": np.asarray(t_fc_b, np.float32).reshape(H, 1).copy(),
        "depT": np.ascontiguousarray(np.asarray(dep_emb, np.float32).T),
        "clbT": np.asarray(cls_b, np.float32).reshape(C, 1).copy(),
    }
    # repmat[c, 16g+c'] = (c' == c)
    rm = np.zeros((C, 128), dtype=np.float32)
    for g in range(8):
        rm[np.arange(C), 16 * g + np.arange(C)] = 1.0
    consts["repmat"] = rm

    if _PROGRAM is None:
        _PROGRAM = build_program()
    nc = _PROGRAM

    in_maps = [_marshal_core(n, input_tensor, dg, consts) for n in range(NCORES)]
    trace = bool(int(os.environ.get("KERNEL_PROFILE", "0")))
    res = run_bass_kernel_spmd(
        nc, in_maps, core_ids=list(range(NCORES)), trace=trace
    )
    if trace and res.exec_time_ns is not None:
        print(f"HW exec time: {res.exec_time_ns} ns")

    out = np.empty((B, L, C, L), dtype=out_dtype)
    for n in range(NCORES):
        b, half = n // 2, n % 2
        i0 = half * RPC
        # raw flat row = (2u+h)*128 + 16g + c ; i_loc = 16u + 8h + g
        raw = res.results[n]["out"].reshape(2 * NINST, 8, 16, L)  # [block=2u+h, g, c16, j]
        out[b, i0:i0 + RPC] = raw[:, :, :C, :].reshape(2 * NINST * 8, C, L)[_ROWPERM]
    return out
